# revision 28
# baseline (speedup 1.0000x reference)
"""Trainium2 Bass kernel for nn_DenseExpert (soft-gated mixture of dense experts).

Math:  out[b,u] = sum_e gate[b,e] * (x[b,:] @ alpha[e]) [u] + (gate @ beta)[b,u]

Strategy (pure data parallel over batch, 8 cores). Per 512-row chunk per core:
  1. DMA x/gate chunk (batch-major); cast to fp16 (11-bit mantissa, close to
     TF32 precision; PE streams fp16 at 1 cycle/row; PSUM accumulation fp32).
  2. Build K=64 block-diagonal gate matrices: dstack[p, e, c] =
     gate[p,e]*[c == p%64], one fp16 tensor_tensor per 128-row tile
     (ident-pattern * gate broadcast, FD=512) on DVE.
  3. y_e.T tiles via PE matmuls: for each 64-row block l,
     yT[i, (e, c)] = x[64l:64l+64, :].T @ dstack[64l:64l+64]   (N=512).
     This replaces both a scale stage and per-expert PE transposes.
  4. PSUM->SBUF copies gather yT into [i, e, b] fp16 layout (DVE/ACT split).
  5. PE matmuls accumulate out.T[u,b] = sum_e alpha_e.T @ y_e.T  plus the
     bias as one K=8 matmul beta.T @ gate.T (gate.T via 4 tiny identity
     matmuls).
  6. out.T (fp32) copied to SBUF and DMA'd to DRAM in [U, B] layout; the
     host does the final cheap transpose when assembling the full result.
"""

import dataclasses
from contextlib import ExitStack

import numpy as np

import concourse.bacc as bacc
import concourse.tile as tile
import concourse.mybir as mybir
from concourse.bass_utils import run_bass_kernel_spmd

F32 = mybir.dt.float32
F16 = mybir.dt.float16

B, E, I, U = 65536, 8, 128, 128
NCORES = 8
BLOC = B // NCORES        # 8192 batch rows per core
CHUNK = 512               # batch rows per pipeline chunk
NCHUNK = BLOC // CHUNK    # 16
TPC = CHUNK // 128        # 128-row tiles per chunk
KB = 64                   # contraction block for the diag trick


def _build():
    nc = bacc.Bacc("TRN2", target_bir_lowering=False, debug=False)

    x = nc.dram_tensor("x", [BLOC, I], F32, kind="ExternalInput").ap()
    gate = nc.dram_tensor("gate", [BLOC, E], F32, kind="ExternalInput").ap()
    alpha = nc.dram_tensor("alpha", [E, I, U], F32, kind="ExternalInput").ap()
    beta = nc.dram_tensor("beta", [E, U], F32, kind="ExternalInput").ap()
    ident = nc.dram_tensor("ident", [128, 128], F16, kind="ExternalInput").ap()
    idrep = nc.dram_tensor("idrep", [128, E, KB], F16, kind="ExternalInput").ap()
    # output stays feature-major on HW; host transposes when assembling
    outT = nc.dram_tensor("outT", [U, BLOC], F32, kind="ExternalOutput").ap()

    with tile.TileContext(nc) as tc, ExitStack() as ctx:
        const = ctx.enter_context(tc.tile_pool(name="const", bufs=1))
        xp = ctx.enter_context(tc.tile_pool(name="xp", bufs=6))
        dgp = ctx.enter_context(tc.tile_pool(name="dgp", bufs=8))
        ytp = ctx.enter_context(tc.tile_pool(name="ytp", bufs=4))
        op = ctx.enter_context(tc.tile_pool(name="op", bufs=3))
        gp = ctx.enter_context(tc.tile_pool(name="gp", bufs=3))
        ps_yt = ctx.enter_context(tc.tile_pool(name="ps_yt", bufs=3, space="PSUM"))
        ps_ot = ctx.enter_context(tc.tile_pool(name="ps_ot", bufs=1, space="PSUM"))
        ps_gt = ctx.enter_context(tc.tile_pool(name="ps_gt", bufs=1, space="PSUM"))

        # --- constants (cast alpha/beta to fp16 on chip) ---
        alpha_sb = const.tile([128, E, U], F32, tag="alpha")
        nc.sync.dma_start(alpha_sb[:], alpha.rearrange("e i u -> i e u"))
        alpha_h = const.tile([128, E, U], F16, tag="alphah")
        nc.vector.tensor_copy(alpha_h[:], alpha_sb[:])

        beta_sb = const.tile([8, U], F32, tag="beta")
        nc.sync.dma_start(beta_sb[:], beta)
        beta_h = const.tile([8, U], F16, tag="betah")
        nc.vector.tensor_copy(beta_h[:], beta_sb[:])

        ident_h = const.tile([128, 128], F16, tag="identh")
        nc.sync.dma_start(ident_h[:], ident)
        idrep_h = const.tile([128, E, KB], F16, tag="idreph")
        nc.sync.dma_start(idrep_h[:], idrep)

        def emit_front(c):
            row0 = c * CHUNK
            g_sb = xp.tile([128, TPC, E], F32, tag="g")
            nc.sync.dma_start(
                g_sb[:],
                gate[row0 : row0 + CHUNK, :].rearrange("(t p) e -> p t e", p=128),
            )
            # x: SWDGE DMA with fused fp32->fp16 cast (issued from GpSimd,
            # which is otherwise idle)
            x_h = xp.tile([128, TPC, I], F16, tag="xh")
            nc.gpsimd.dma_start(
                x_h[:], x[row0 : row0 + CHUNK, :].rearrange("(t p) i -> p t i", p=128)
            )
            g_h = xp.tile([128, TPC, E], F16, tag="gh")
            nc.vector.tensor_copy(g_h[:], g_sb[:])

            # gate.T for the bias matmul, via tiny identity matmuls
            gT_ps = ps_gt.tile([E, TPC, 128], F32, tag="gTps")
            for t in range(TPC):
                nc.tensor.matmul(
                    gT_ps[:, t, :], g_h[:, t, :], ident_h[:], start=True, stop=True
                )
            gT_h = gp.tile([E, TPC, 128], F16, tag="gTh")
            nc.vector.tensor_copy(gT_h[:], gT_ps[:])

            # per 128-row tile: diag build (DVE) + yT matmuls + gather copy
            yT_all = ytp.tile([128, E, TPC, 128], F16, tag="yT")
            for t in range(TPC):
                gview = dataclasses.replace(
                    g_h[:],
                    ap=[[TPC * E, 128], [1, E], [0, KB]],
                    offset=t * E,
                )
                diag = dgp.tile([128, E, KB], F16, tag="diag")
                nc.vector.tensor_tensor(
                    diag[:], gview, idrep_h[:], op=mybir.AluOpType.mult
                )
                yT_ps = ps_yt.tile([128, 2, E, KB], F32, tag="yTps")
                for l in range(2):
                    nc.tensor.matmul(
                        yT_ps[:, l, :, :],
                        x_h[l * KB : (l + 1) * KB, t, :],
                        diag[l * KB : (l + 1) * KB, :, :],
                        start=True,
                        stop=True,
                    )
                dst = dataclasses.replace(
                    yT_all[:],
                    ap=[[E * TPC * 128, 128], [KB, 2], [TPC * 128, E], [1, KB]],
                    offset=t * 128,
                )
                if t == 3:
                    nc.vector.tensor_copy(dst, yT_ps[:])
                else:
                    nc.scalar.copy(dst, yT_ps[:])
            return yT_all, gT_h

        def emit_back(c, yT_all, gT_h):
            row0 = c * CHUNK
            oT_ps = ps_ot.tile([128, CHUNK], F32, tag="oTps")
            for e in range(E):
                nc.tensor.matmul(
                    oT_ps[:],
                    alpha_h[:, e, :],
                    yT_all[:, e, :, :],
                    start=(e == 0),
                    stop=False,
                )
            nc.tensor.matmul(oT_ps[:], beta_h[:], gT_h[:], start=False, stop=True)

            oT_sb = op.tile([128, CHUNK], F32, tag="oT")
            nc.vector.tensor_copy(oT_sb[:, : CHUNK // 2], oT_ps[:, : CHUNK // 2])
            nc.scalar.copy(oT_sb[:, CHUNK // 2 :], oT_ps[:, CHUNK // 2 :])
            nc.sync.dma_start(outT[:, row0 : row0 + CHUNK], oT_sb[:])

        pending = None
        for c in range(NCHUNK):
            front = emit_front(c)
            if pending is not None:
                emit_back(c - 1, *pending)
            pending = front
        emit_back(NCHUNK - 1, *pending)

    nc.compile()
    return nc


_NC_CACHE = None


def _make_idrep():
    idrep = np.zeros((128, E, KB), np.float16)
    for p in range(128):
        idrep[p, :, p % KB] = 1.0
    return idrep


def make_in_maps(x, gate_perc, alpha, beta):
    x = np.ascontiguousarray(np.asarray(x, dtype=np.float32))
    gate_perc = np.ascontiguousarray(np.asarray(gate_perc, dtype=np.float32))
    alpha = np.ascontiguousarray(np.asarray(alpha, dtype=np.float32))
    beta = np.ascontiguousarray(np.asarray(beta, dtype=np.float32))
    ident = np.eye(128, dtype=np.float16)
    idrep = _make_idrep()
    in_maps = []
    for c in range(NCORES):
        sl = slice(c * BLOC, (c + 1) * BLOC)
        in_maps.append(
            {
                "x": x[sl],
                "gate": gate_perc[sl],
                "alpha": alpha,
                "beta": beta,
                "ident": ident,
                "idrep": idrep,
            }
        )
    return in_maps


def kernel(x, gate_perc, alpha, beta):
    global _NC_CACHE
    if _NC_CACHE is None:
        _NC_CACHE = _build()
    nc = _NC_CACHE

    in_maps = make_in_maps(x, gate_perc, alpha, beta)
    res = run_bass_kernel_spmd(nc, in_maps, list(range(NCORES))).results
    # per-core outputs are [U, BLOC]; assemble and transpose on host
    full_T = np.concatenate([res[c]["outT"] for c in range(NCORES)], axis=1)
    return np.ascontiguousarray(full_T.T)


if __name__ == "__main__":
    rng = np.random.default_rng(0)
    x = rng.standard_normal((B, I)).astype(np.float32)
    g = rng.random((B, E)).astype(np.float32)
    g /= g.sum(-1, keepdims=True)
    al = (rng.standard_normal((E, I, U)) * 0.05).astype(np.float32)
    be = (rng.standard_normal((E, U)) * 0.05).astype(np.float32)
    got = kernel(x, g, al, be)
    ref = np.einsum("bi,eio->beo", x, al, optimize=True)
    ref = np.einsum("beo,be->bo", ref, g) + g @ be
    err = np.abs(got - ref)
    print("max abs err", err.max(), "rel", err.max() / np.abs(ref).max())


# revision 29
# speedup vs baseline: 1.0247x; 1.0247x over previous
"""Trainium2 Bass kernel for nn_DenseExpert (soft-gated mixture of dense experts).

Math:  out[b,u] = sum_e gate[b,e] * (x[b,:] @ alpha[e]) [u] + (gate @ beta)[b,u]

Strategy (pure data parallel over batch, 8 cores). Per 512-row chunk per core:
  1. DMA x/gate chunk (batch-major); cast to fp16 (11-bit mantissa, close to
     TF32 precision; PE streams fp16 at 1 cycle/row; PSUM accumulation fp32).
  2. Build K=64 block-diagonal gate matrices: dstack[p, e, c] =
     gate[p,e]*[c == p%64], one fp16 tensor_tensor per 128-row tile
     (ident-pattern * gate broadcast, FD=512) on DVE.
  3. y_e.T tiles via PE matmuls: for each 64-row block l,
     yT[i, (e, c)] = x[64l:64l+64, :].T @ dstack[64l:64l+64]   (N=512).
     This replaces both a scale stage and per-expert PE transposes.
  4. PSUM->SBUF copies gather yT into [i, e, b] fp16 layout (DVE/ACT split).
  5. PE matmuls accumulate out.T[u,b] = sum_e alpha_e.T @ y_e.T  plus the
     bias as one K=8 matmul beta.T @ gate.T (gate.T via 4 tiny identity
     matmuls).
  6. out.T (fp32) copied to SBUF and DMA'd to DRAM in [U, B] layout; the
     host does the final cheap transpose when assembling the full result.
"""

import dataclasses
from contextlib import ExitStack

import numpy as np

import concourse.bacc as bacc
import concourse.tile as tile
import concourse.mybir as mybir
from concourse.bass_utils import run_bass_kernel_spmd

F32 = mybir.dt.float32
F16 = mybir.dt.float16

B, E, I, U = 65536, 8, 128, 128
NCORES = 8
BLOC = B // NCORES        # 8192 batch rows per core
CHUNK = 512               # batch rows per pipeline chunk
NCHUNK = BLOC // CHUNK    # 16
TPC = CHUNK // 128        # 128-row tiles per chunk
KB = 64                   # contraction block for the diag trick


def _build():
    nc = bacc.Bacc("TRN2", target_bir_lowering=False, debug=False)

    x = nc.dram_tensor("x", [BLOC, I], F32, kind="ExternalInput").ap()
    gate = nc.dram_tensor("gate", [BLOC, E], F32, kind="ExternalInput").ap()
    alpha = nc.dram_tensor("alpha", [E, I, U], F32, kind="ExternalInput").ap()
    beta = nc.dram_tensor("beta", [E, U], F32, kind="ExternalInput").ap()
    ident = nc.dram_tensor("ident", [128, 128], F16, kind="ExternalInput").ap()
    idrep = nc.dram_tensor("idrep", [128, E, KB], F16, kind="ExternalInput").ap()
    # output stays feature-major on HW; host transposes when assembling
    outT = nc.dram_tensor("outT", [U, BLOC], F32, kind="ExternalOutput").ap()

    with tile.TileContext(nc) as tc, ExitStack() as ctx:
        const = ctx.enter_context(tc.tile_pool(name="const", bufs=1))
        xp = ctx.enter_context(tc.tile_pool(name="xp", bufs=6))
        dgp = ctx.enter_context(tc.tile_pool(name="dgp", bufs=8))
        ytp = ctx.enter_context(tc.tile_pool(name="ytp", bufs=4))
        op = ctx.enter_context(tc.tile_pool(name="op", bufs=3))
        gp = ctx.enter_context(tc.tile_pool(name="gp", bufs=3))
        ps_yt = ctx.enter_context(tc.tile_pool(name="ps_yt", bufs=3, space="PSUM"))
        ps_ot = ctx.enter_context(tc.tile_pool(name="ps_ot", bufs=1, space="PSUM"))
        ps_gt = ctx.enter_context(tc.tile_pool(name="ps_gt", bufs=1, space="PSUM"))

        # --- constants (cast alpha/beta to fp16 on chip) ---
        alpha_sb = const.tile([128, E, U], F32, tag="alpha")
        nc.sync.dma_start(alpha_sb[:], alpha.rearrange("e i u -> i e u"))
        alpha_h = const.tile([128, E, U], F16, tag="alphah")
        nc.vector.tensor_copy(alpha_h[:], alpha_sb[:])

        beta_sb = const.tile([8, U], F32, tag="beta")
        nc.sync.dma_start(beta_sb[:], beta)
        beta_h = const.tile([8, U], F16, tag="betah")
        nc.vector.tensor_copy(beta_h[:], beta_sb[:])

        ident_h = const.tile([128, 128], F16, tag="identh")
        nc.sync.dma_start(ident_h[:], ident)
        idrep_h = const.tile([128, E, KB], F16, tag="idreph")
        nc.sync.dma_start(idrep_h[:], idrep)

        def emit_front(c):
            row0 = c * CHUNK
            g_sb = xp.tile([128, TPC, E], F32, tag="g")
            nc.sync.dma_start(
                g_sb[:],
                gate[row0 : row0 + CHUNK, :].rearrange("(t p) e -> p t e", p=128),
            )
            # x: SWDGE DMA with fused fp32->fp16 cast (issued from GpSimd,
            # which is otherwise idle)
            x_h = xp.tile([128, TPC, I], F16, tag="xh")
            nc.gpsimd.dma_start(
                x_h[:], x[row0 : row0 + CHUNK, :].rearrange("(t p) i -> p t i", p=128)
            )
            g_h = xp.tile([128, TPC, E], F16, tag="gh")
            nc.vector.tensor_copy(g_h[:], g_sb[:])

            # gate.T for the bias matmul, via tiny identity matmuls
            gT_ps = ps_gt.tile([E, TPC, 128], F32, tag="gTps")
            for t in range(TPC):
                nc.tensor.matmul(
                    gT_ps[:, t, :], g_h[:, t, :], ident_h[:], start=True, stop=True
                )
            gT_h = gp.tile([E, TPC, 128], F16, tag="gTh")
            nc.vector.tensor_copy(gT_h[:], gT_ps[:])

            # per 128-row tile: diag build (DVE) + yT matmuls + gather copy
            yT_all = ytp.tile([128, E, TPC, 128], F16, tag="yT")
            for t in range(TPC):
                gview = dataclasses.replace(
                    g_h[:],
                    ap=[[TPC * E, 128], [1, E], [0, KB]],
                    offset=t * E,
                )
                diag = dgp.tile([128, E, KB], F16, tag="diag")
                nc.vector.tensor_tensor(
                    diag[:], idrep_h[:], gview, op=mybir.AluOpType.mult
                )
                yT_ps = ps_yt.tile([128, 2, E, KB], F32, tag="yTps")
                for l in range(2):
                    nc.tensor.matmul(
                        yT_ps[:, l, :, :],
                        x_h[l * KB : (l + 1) * KB, t, :],
                        diag[l * KB : (l + 1) * KB, :, :],
                        start=True,
                        stop=True,
                    )
                dst = dataclasses.replace(
                    yT_all[:],
                    ap=[[E * TPC * 128, 128], [KB, 2], [TPC * 128, E], [1, KB]],
                    offset=t * 128,
                )
                if t == 3:
                    nc.vector.tensor_copy(dst, yT_ps[:])
                else:
                    nc.scalar.copy(dst, yT_ps[:])
            return yT_all, gT_h

        def emit_back(c, yT_all, gT_h):
            row0 = c * CHUNK
            oT_ps = ps_ot.tile([128, CHUNK], F32, tag="oTps")
            for e in range(E):
                nc.tensor.matmul(
                    oT_ps[:],
                    alpha_h[:, e, :],
                    yT_all[:, e, :, :],
                    start=(e == 0),
                    stop=False,
                )
            nc.tensor.matmul(oT_ps[:], beta_h[:], gT_h[:], start=False, stop=True)

            oT_sb = op.tile([128, CHUNK], F32, tag="oT")
            nc.vector.tensor_copy(oT_sb[:, : CHUNK // 2], oT_ps[:, : CHUNK // 2])
            nc.scalar.copy(oT_sb[:, CHUNK // 2 :], oT_ps[:, CHUNK // 2 :])
            nc.sync.dma_start(outT[:, row0 : row0 + CHUNK], oT_sb[:])

        pending = None
        for c in range(NCHUNK):
            front = emit_front(c)
            if pending is not None:
                emit_back(c - 1, *pending)
            pending = front
        emit_back(NCHUNK - 1, *pending)

    nc.compile()
    return nc


_NC_CACHE = None


def _make_idrep():
    idrep = np.zeros((128, E, KB), np.float16)
    for p in range(128):
        idrep[p, :, p % KB] = 1.0
    return idrep


def make_in_maps(x, gate_perc, alpha, beta):
    x = np.ascontiguousarray(np.asarray(x, dtype=np.float32))
    gate_perc = np.ascontiguousarray(np.asarray(gate_perc, dtype=np.float32))
    alpha = np.ascontiguousarray(np.asarray(alpha, dtype=np.float32))
    beta = np.ascontiguousarray(np.asarray(beta, dtype=np.float32))
    ident = np.eye(128, dtype=np.float16)
    idrep = _make_idrep()
    in_maps = []
    for c in range(NCORES):
        sl = slice(c * BLOC, (c + 1) * BLOC)
        in_maps.append(
            {
                "x": x[sl],
                "gate": gate_perc[sl],
                "alpha": alpha,
                "beta": beta,
                "ident": ident,
                "idrep": idrep,
            }
        )
    return in_maps


def kernel(x, gate_perc, alpha, beta):
    global _NC_CACHE
    if _NC_CACHE is None:
        _NC_CACHE = _build()
    nc = _NC_CACHE

    in_maps = make_in_maps(x, gate_perc, alpha, beta)
    res = run_bass_kernel_spmd(nc, in_maps, list(range(NCORES))).results
    # per-core outputs are [U, BLOC]; assemble and transpose on host
    full_T = np.concatenate([res[c]["outT"] for c in range(NCORES)], axis=1)
    return np.ascontiguousarray(full_T.T)


if __name__ == "__main__":
    rng = np.random.default_rng(0)
    x = rng.standard_normal((B, I)).astype(np.float32)
    g = rng.random((B, E)).astype(np.float32)
    g /= g.sum(-1, keepdims=True)
    al = (rng.standard_normal((E, I, U)) * 0.05).astype(np.float32)
    be = (rng.standard_normal((E, U)) * 0.05).astype(np.float32)
    got = kernel(x, g, al, be)
    ref = np.einsum("bi,eio->beo", x, al, optimize=True)
    ref = np.einsum("beo,be->bo", ref, g) + g @ be
    err = np.abs(got - ref)
    print("max abs err", err.max(), "rel", err.max() / np.abs(ref).max())


# revision 30
# speedup vs baseline: 1.1071x; 1.0803x over previous
"""Trainium2 Bass kernel for nn_DenseExpert (soft-gated mixture of dense experts).

Math:  out[b,u] = sum_e gate[b,e] * (x[b,:] @ alpha[e]) [u] + (gate @ beta)[b,u]

Strategy (pure data parallel over batch, 8 cores). Per 512-row chunk per core:
  1. DMA x/gate chunk (batch-major); cast to fp16 (11-bit mantissa, close to
     TF32 precision; PE streams fp16 at 1 cycle/row; PSUM accumulation fp32).
  2. Build K=64 block-diagonal gate matrices: dstack[p, e, c] =
     gate[p,e]*[c == p%64], one fp16 tensor_tensor per 128-row tile
     (ident-pattern * gate broadcast, FD=512) on DVE.
  3. y_e.T tiles via PE matmuls: for each 64-row block l,
     yT[i, (e, c)] = x[64l:64l+64, :].T @ dstack[64l:64l+64]   (N=512).
     This replaces both a scale stage and per-expert PE transposes.
  4. PSUM->SBUF copies gather yT into [i, e, b] fp16 layout (DVE/ACT split).
  5. PE matmuls accumulate out.T[u,b] = sum_e alpha_e.T @ y_e.T  plus the
     bias as one K=8 matmul beta.T @ gate.T (gate.T via 4 tiny identity
     matmuls).
  6. out.T (fp32) copied to SBUF and DMA'd to DRAM in [U, B] layout; the
     host does the final cheap transpose when assembling the full result.
"""

import dataclasses
from contextlib import ExitStack

import numpy as np

import concourse.bacc as bacc
import concourse.tile as tile
import concourse.mybir as mybir
from concourse.bass_utils import run_bass_kernel_spmd

F32 = mybir.dt.float32
F16 = mybir.dt.float16

B, E, I, U = 65536, 8, 128, 128
NCORES = 8
BLOC = B // NCORES        # 8192 batch rows per core
CHUNK = 512               # batch rows per pipeline chunk
NCHUNK = BLOC // CHUNK    # 16
TPC = CHUNK // 128        # 128-row tiles per chunk
KB = 64                   # contraction block for the diag trick


def _build():
    nc = bacc.Bacc("TRN2", target_bir_lowering=False, debug=False)

    x = nc.dram_tensor("x", [BLOC, I], F32, kind="ExternalInput").ap()
    gate = nc.dram_tensor("gate", [BLOC, E], F32, kind="ExternalInput").ap()
    alpha = nc.dram_tensor("alpha", [E, I, U], F32, kind="ExternalInput").ap()
    beta = nc.dram_tensor("beta", [E, U], F32, kind="ExternalInput").ap()
    ident = nc.dram_tensor("ident", [128, 128], F16, kind="ExternalInput").ap()
    idrep = nc.dram_tensor("idrep", [128, E, KB], F16, kind="ExternalInput").ap()
    # output stays feature-major on HW; host transposes when assembling
    outT = nc.dram_tensor("outT", [U, BLOC], F32, kind="ExternalOutput").ap()

    with tile.TileContext(nc) as tc, ExitStack() as ctx:
        const = ctx.enter_context(tc.tile_pool(name="const", bufs=1))
        xp = ctx.enter_context(tc.tile_pool(name="xp", bufs=6))
        dgp = ctx.enter_context(tc.tile_pool(name="dgp", bufs=8))
        ytp = ctx.enter_context(tc.tile_pool(name="ytp", bufs=4))
        op = ctx.enter_context(tc.tile_pool(name="op", bufs=3))
        gp = ctx.enter_context(tc.tile_pool(name="gp", bufs=3))
        ps_yt = ctx.enter_context(tc.tile_pool(name="ps_yt", bufs=3, space="PSUM"))
        ps_ot = ctx.enter_context(tc.tile_pool(name="ps_ot", bufs=1, space="PSUM"))
        ps_gt = ctx.enter_context(tc.tile_pool(name="ps_gt", bufs=1, space="PSUM"))

        # --- constants (cast alpha/beta to fp16 on chip) ---
        alpha_sb = const.tile([128, E, U], F32, tag="alpha")
        nc.sync.dma_start(alpha_sb[:], alpha.rearrange("e i u -> i e u"))
        alpha_h = const.tile([128, E, U], F16, tag="alphah")
        nc.vector.tensor_copy(alpha_h[:], alpha_sb[:])

        beta_sb = const.tile([8, U], F32, tag="beta")
        nc.sync.dma_start(beta_sb[:], beta)
        beta_h = const.tile([8, U], F16, tag="betah")
        nc.vector.tensor_copy(beta_h[:], beta_sb[:])

        ident_h = const.tile([128, 128], F16, tag="identh")
        nc.sync.dma_start(ident_h[:], ident)
        idrep_h = const.tile([128, E, KB], F16, tag="idreph")
        nc.sync.dma_start(idrep_h[:], idrep)

        def emit_front(c):
            row0 = c * CHUNK
            g_sb = xp.tile([128, TPC, E], F32, tag="g")
            nc.sync.dma_start(
                g_sb[:],
                gate[row0 : row0 + CHUNK, :].rearrange("(t p) e -> p t e", p=128),
            )
            # x: SWDGE DMA with fused fp32->fp16 cast (issued from GpSimd,
            # which is otherwise idle)
            x_h = xp.tile([128, TPC, I], F16, tag="xh")
            nc.gpsimd.dma_start(
                x_h[:], x[row0 : row0 + CHUNK, :].rearrange("(t p) i -> p t i", p=128)
            )
            g_h = xp.tile([128, TPC, E], F16, tag="gh")
            nc.vector.tensor_copy(g_h[:], g_sb[:])

            # gate.T for the bias matmul, via tiny identity matmuls
            gT_ps = ps_gt.tile([E, TPC, 128], F32, tag="gTps")
            for t in range(TPC):
                nc.tensor.matmul(
                    gT_ps[:, t, :], g_h[:, t, :], ident_h[:], start=True, stop=True
                )
            gT_h = gp.tile([E, TPC, 128], F16, tag="gTh")
            nc.vector.tensor_copy(gT_h[:], gT_ps[:])

            # per 128-row tile: diag build (DVE) + yT matmuls + gather copy
            yT_all = ytp.tile([128, E, TPC, 128], F16, tag="yT")
            for t in range(TPC):
                gview = dataclasses.replace(
                    g_h[:],
                    ap=[[TPC * E, 128], [1, E], [0, KB]],
                    offset=t * E,
                )
                diag = dgp.tile([128, E, KB], F16, tag="diag")
                nc.vector.tensor_tensor(
                    diag[:], idrep_h[:], gview, op=mybir.AluOpType.mult
                )
                yT_ps = ps_yt.tile([128, 2, E, KB], F32, tag="yTps")
                for l in range(2):
                    nc.tensor.matmul(
                        yT_ps[:, l, :, :],
                        x_h[l * KB : (l + 1) * KB, t, :],
                        diag[l * KB : (l + 1) * KB, :, :],
                        start=True,
                        stop=True,
                    )
                dst = dataclasses.replace(
                    yT_all[:],
                    ap=[[E * TPC * 128, 128], [KB, 2], [TPC * 128, E], [1, KB]],
                    offset=t * 128,
                )
                if t == 3:
                    # split the last tile's gather across DVE and ACT
                    dst0 = dataclasses.replace(
                        yT_all[:],
                        ap=[[E * TPC * 128, 128], [TPC * 128, E], [1, KB]],
                        offset=t * 128,
                    )
                    dst1 = dataclasses.replace(
                        yT_all[:],
                        ap=[[E * TPC * 128, 128], [TPC * 128, E], [1, KB]],
                        offset=t * 128 + KB,
                    )
                    nc.vector.tensor_copy(dst0, yT_ps[:, 0, :, :])
                    nc.scalar.copy(dst1, yT_ps[:, 1, :, :])
                else:
                    nc.scalar.copy(dst, yT_ps[:])
            return yT_all, gT_h

        def emit_back(c, yT_all, gT_h):
            row0 = c * CHUNK
            oT_ps = ps_ot.tile([128, CHUNK], F32, tag="oTps")
            for e in range(E):
                nc.tensor.matmul(
                    oT_ps[:],
                    alpha_h[:, e, :],
                    yT_all[:, e, :, :],
                    start=(e == 0),
                    stop=False,
                )
            nc.tensor.matmul(oT_ps[:], beta_h[:], gT_h[:], start=False, stop=True)

            oT_sb = op.tile([128, CHUNK], F32, tag="oT")
            nc.vector.tensor_copy(oT_sb[:, : CHUNK // 2], oT_ps[:, : CHUNK // 2])
            nc.scalar.copy(oT_sb[:, CHUNK // 2 :], oT_ps[:, CHUNK // 2 :])
            nc.sync.dma_start(outT[:, row0 : row0 + CHUNK], oT_sb[:])

        pending = None
        for c in range(NCHUNK):
            front = emit_front(c)
            if pending is not None:
                emit_back(c - 1, *pending)
            pending = front
        emit_back(NCHUNK - 1, *pending)

    nc.compile()
    return nc


_NC_CACHE = None


def _make_idrep():
    idrep = np.zeros((128, E, KB), np.float16)
    for p in range(128):
        idrep[p, :, p % KB] = 1.0
    return idrep


def make_in_maps(x, gate_perc, alpha, beta):
    x = np.ascontiguousarray(np.asarray(x, dtype=np.float32))
    gate_perc = np.ascontiguousarray(np.asarray(gate_perc, dtype=np.float32))
    alpha = np.ascontiguousarray(np.asarray(alpha, dtype=np.float32))
    beta = np.ascontiguousarray(np.asarray(beta, dtype=np.float32))
    ident = np.eye(128, dtype=np.float16)
    idrep = _make_idrep()
    in_maps = []
    for c in range(NCORES):
        sl = slice(c * BLOC, (c + 1) * BLOC)
        in_maps.append(
            {
                "x": x[sl],
                "gate": gate_perc[sl],
                "alpha": alpha,
                "beta": beta,
                "ident": ident,
                "idrep": idrep,
            }
        )
    return in_maps


def kernel(x, gate_perc, alpha, beta):
    global _NC_CACHE
    if _NC_CACHE is None:
        _NC_CACHE = _build()
    nc = _NC_CACHE

    in_maps = make_in_maps(x, gate_perc, alpha, beta)
    res = run_bass_kernel_spmd(nc, in_maps, list(range(NCORES))).results
    # per-core outputs are [U, BLOC]; assemble and transpose on host
    full_T = np.concatenate([res[c]["outT"] for c in range(NCORES)], axis=1)
    return np.ascontiguousarray(full_T.T)


if __name__ == "__main__":
    rng = np.random.default_rng(0)
    x = rng.standard_normal((B, I)).astype(np.float32)
    g = rng.random((B, E)).astype(np.float32)
    g /= g.sum(-1, keepdims=True)
    al = (rng.standard_normal((E, I, U)) * 0.05).astype(np.float32)
    be = (rng.standard_normal((E, U)) * 0.05).astype(np.float32)
    got = kernel(x, g, al, be)
    ref = np.einsum("bi,eio->beo", x, al, optimize=True)
    ref = np.einsum("beo,be->bo", ref, g) + g @ be
    err = np.abs(got - ref)
    print("max abs err", err.max(), "rel", err.max() / np.abs(ref).max())
